# revision 1
# baseline (speedup 1.0000x reference)
"""Data-parallel GeneratedTreeClassifier forward for 8 NeuronCores.

Shards the batch dim of x (16384 -> 8 x 2048) across cores, replicates
the small tree params/weights, computes the soft-decision-tree forward
on each core, and gathers the full [16384, 512] output.
"""
import numpy as np
import jax
import jax.numpy as jnp
from functools import partial

INPUT_DIM = 512
N_CLASSES = 512
N_TREES = 64
TREE_DEPTH = 3
N_LEAVES = 2 ** TREE_DEPTH            # 8
N_INTERNAL = 2 ** TREE_DEPTH - 1      # 7
PARAM_PER_TREE = N_INTERNAL * (INPUT_DIM + 1) + N_LEAVES * N_CLASSES
BATCH = 16384
N_CORES = 8
EPS = 1e-8


def _forward_shard(x, tree_params, tree_weights):
    # x: [B/8, D]; tree_params: [1, T*PPT]; tree_weights: [1, T]
    p = tree_params[0].reshape(N_TREES, PARAM_PER_TREE)
    nw = N_INTERNAL * INPUT_DIM
    split_w = p[:, :nw].reshape(N_TREES * N_INTERNAL, INPUT_DIM)       # [T*I, D]
    split_b = p[:, nw:nw + N_INTERNAL].reshape(N_TREES * N_INTERNAL)   # [T*I]
    leaf_logits = p[:, nw + N_INTERNAL:].reshape(N_TREES, N_LEAVES, N_CLASSES)

    b = x.shape[0]
    dec = jax.nn.sigmoid(x @ split_w.T + split_b).reshape(b, N_TREES, N_INTERNAL)

    d0 = dec[:, :, 0]
    d1 = dec[:, :, 1]
    d2 = dec[:, :, 2]
    d3 = dec[:, :, 3]
    one = jnp.ones_like(d0)
    # leaf order per reference loop: [1, d0, 1-d0, d1, 1-d1, d2, 1-d2, d3]
    leaf = jnp.stack(
        [one, d0, 1.0 - d0, d1, 1.0 - d1, d2, 1.0 - d2, d3], axis=-1
    )  # [b, T, L]
    leaf = leaf / (leaf.sum(axis=-1, keepdims=True) + EPS)

    leaf_dist = jax.nn.softmax(leaf_logits, axis=-1)                   # [T, L, C]
    m = (tree_weights[0][:, None, None] * leaf_dist).reshape(
        N_TREES * N_LEAVES, N_CLASSES
    )
    return leaf.reshape(b, N_TREES * N_LEAVES) @ m                     # [b, C]


_pmapped = jax.pmap(_forward_shard, in_axes=(0, None, None))


def kernel(x: np.ndarray, tree_params: np.ndarray, tree_weights: np.ndarray) -> np.ndarray:
    xs = np.asarray(x, dtype=np.float32).reshape(N_CORES, BATCH // N_CORES, INPUT_DIM)
    out = _pmapped(
        xs,
        jnp.asarray(tree_params, dtype=jnp.float32),
        jnp.asarray(tree_weights, dtype=jnp.float32),
    )
    return np.asarray(out).reshape(BATCH, N_CLASSES)



# revision 2
# speedup vs baseline: 1568.5112x; 1568.5112x over previous
"""Data-parallel GeneratedTreeClassifier forward on 8 NeuronCores (Bass/Tile).

Shards the batch dim of x (16384 -> 8 x 2048) across cores, replicates the
small tree params, runs a hand-written Bass/Tile kernel per core, and
gathers the full [16384, 512] output.

Per-core device graph (batch tile = 128 rows, 16 tiles):
  xT   <- DMA-transpose of bf16 x tile            (no PE/ACT cost)
  z    = xT.T @ W^T + bias  (PE, bf16, K=512)     -> PSUM [128, 256]
  d    = sigmoid(z)          (ACT)
  r    = 1 / (4 + d3 + eps)  (DVE)   [leaf sum is 4 + d3 algebraically]
  leaf = [r, d0*r, r-d0*r, d1*r, r-d1*r, d2*r, r-d2*r, d3*r]  (DVE, bf16)
  lfT  <- DMA-transpose of leaf
  out  = lfT.T @ M           (PE, bf16, K=512)    -> PSUM [128, 512]
  M    = softmax(leaf_logits) * tree_weight  (one-time, ACT/DVE on device)
"""
import numpy as np
import ml_dtypes
from contextlib import ExitStack

import concourse.bass as bass
import concourse.tile as tile
from concourse import bacc, mybir

INPUT_DIM = 512
N_CLASSES = 512
N_TREES = 64
N_LEAVES = 8
N_INTERNAL = 7
PPT = N_INTERNAL * (INPUT_DIM + 1) + N_LEAVES * N_CLASSES
BATCH = 16384
N_CORES = 8
BSH = BATCH // N_CORES          # 2048 rows per core
NB = BSH // 128                 # 16 batch tiles per core
NW = N_INTERNAL * INPUT_DIM
EPS = 1e-8

F32 = mybir.dt.float32
BF16 = mybir.dt.bfloat16


def _emit(ctx: ExitStack, tc, xb, wT, bb, ll, wr, out):
    nc = tc.nc

    const = ctx.enter_context(tc.tile_pool(name="const", bufs=1))

    # Replicated params, resident in SBUF.
    wt_sb = const.tile([128, 4, 256], BF16)
    nc.sync.dma_start(wt_sb[:], wT.rearrange("(k p) j -> p k j", p=128))
    bias_sb = const.tile([1, 256], BF16)
    nc.sync.dma_start(bias_sb[:], bb[:])
    ones_sb = const.tile([1, 128], BF16)
    nc.vector.memset(ones_sb[:], 1.0)
    m_sb = const.tile([128, 4, N_CLASSES], BF16)

    # M = softmax(leaf_logits, axis=-1) * w_tree   (rows tl = t*8 + l)
    ppool = ctx.enter_context(tc.tile_pool(name="prm", bufs=2))
    for k in range(4):
        llt = ppool.tile([128, N_CLASSES], F32, tag="llt")
        nc.sync.dma_start(llt[:], ll[k * 128:(k + 1) * 128, :])
        mx = ppool.tile([128, 1], F32, tag="mx")
        nc.vector.reduce_max(mx[:], llt[:], axis=mybir.AxisListType.X)
        nmx = ppool.tile([128, 1], F32, tag="nmx")
        nc.vector.tensor_scalar_mul(nmx[:], mx[:], -1.0)
        e = ppool.tile([128, N_CLASSES], F32, tag="e")
        s = ppool.tile([128, 1], F32, tag="s")
        nc.scalar.activation(e[:], llt[:], mybir.ActivationFunctionType.Exp,
                             bias=nmx[:], scale=1.0, accum_out=s[:])
        rs = ppool.tile([128, 1], F32, tag="rs")
        nc.vector.reciprocal(rs[:], s[:])
        wrt = ppool.tile([128, 1], F32, tag="wrt")
        nc.sync.dma_start(wrt[:], wr[k * 128:(k + 1) * 128, :])
        sc = ppool.tile([128, 1], F32, tag="sc")
        nc.vector.tensor_tensor(sc[:], rs[:], wrt[:], op=mybir.AluOpType.mult)
        nc.vector.tensor_scalar_mul(m_sb[:, k, :], e[:], sc[:])

    xpool = ctx.enter_context(tc.tile_pool(name="xT", bufs=3))
    lpool = ctx.enter_context(tc.tile_pool(name="lfT", bufs=3))
    dpool = ctx.enter_context(tc.tile_pool(name="work", bufs=3))
    opool = ctx.enter_context(tc.tile_pool(name="osb", bufs=3))
    zpsum = ctx.enter_context(tc.tile_pool(name="zps", bufs=2, space="PSUM"))
    opsum = ctx.enter_context(tc.tile_pool(name="ops", bufs=2, space="PSUM"))

    xv = xb.rearrange("(n p) (k q) -> n p k q", p=128, q=128)   # [16,128,4,128]

    for i in range(NB):
        # x tile transposed: xT[:, k, b] = x[i*128+b, k*128+:]
        xT = xpool.tile([128, 4, 128], BF16, tag="xT")
        for k in range(4):
            nc.sync.dma_start(xT[:, k, :], xv[i, :, k, :], transpose=True)

        # z = x @ W^T + bias   -> PSUM [128b, 256j], j = t*4 + i
        zps = zpsum.tile([128, 256], F32, tag="z")
        for k in range(4):
            nc.tensor.matmul(zps[:], lhsT=xT[:, k, :], rhs=wt_sb[:, k, :],
                             start=(k == 0), stop=False)
        nc.tensor.matmul(zps[:], lhsT=ones_sb[:], rhs=bias_sb[:],
                         start=False, stop=True)

        d = dpool.tile([128, 256], F32, tag="d")
        nc.scalar.activation(d[:], zps[:], mybir.ActivationFunctionType.Sigmoid)
        dv = d[:].rearrange("p (t i) -> p t i", i=4)

        t4 = dpool.tile([128, 64], F32, tag="t4")
        nc.vector.tensor_scalar_add(t4[:], dv[:, :, 3], 4.0 + EPS)
        r = dpool.tile([128, 64], F32, tag="r")
        nc.vector.reciprocal(r[:], t4[:])

        # leaf[p, t, l] with l = 2q + m:
        #   l=0: r    l=1,3,5,7 (m=1): d_i*r    l=2,4,6 (q>=1, m=0): r-d_i*r
        leaf = dpool.tile([128, 64, 4, 2], BF16, tag="leaf")
        rb4 = r[:].unsqueeze(2).broadcast_to([128, 64, 4])
        rb3 = r[:].unsqueeze(2).broadcast_to([128, 64, 3])
        nc.vector.tensor_copy(leaf[:, :, 0, 0], r[:])
        nc.vector.tensor_tensor(leaf[:, :, :, 1], dv[:], rb4,
                                op=mybir.AluOpType.mult)
        nc.vector.tensor_tensor(leaf[:, :, 1:4, 0], rb3, leaf[:, :, 0:3, 1],
                                op=mybir.AluOpType.subtract)

        # leaf transposed: lfT[:, k, b] = leaf_flat[b, k*128+:]
        lfT = lpool.tile([128, 4, 128], BF16, tag="lfT")
        lv = leaf[:].rearrange("p t q m -> p (t q m)")
        for k in range(4):
            nc.sync.dma_start(lfT[:, k, :], lv[:, k * 128:(k + 1) * 128],
                              transpose=True)

        # out = leaf @ M  -> PSUM [128b, 512c]
        ops = opsum.tile([128, 512], F32, tag="o")
        for k in range(4):
            nc.tensor.matmul(ops[:], lhsT=lfT[:, k, :], rhs=m_sb[:, k, :],
                             start=(k == 0), stop=(k == 3))
        osb = opool.tile([128, 512], F32, tag="osb")
        nc.scalar.copy(osb[:], ops[:])
        nc.sync.dma_start(out[i * 128:(i + 1) * 128, :], osb[:])


_NC = None


def _get_nc():
    global _NC
    if _NC is None:
        nc = bacc.Bacc("TRN2", target_bir_lowering=False, debug=False)
        xb = nc.dram_tensor("xb", [BSH, INPUT_DIM], BF16, kind="ExternalInput")
        wT = nc.dram_tensor("wT", [INPUT_DIM, 256], BF16, kind="ExternalInput")
        bb = nc.dram_tensor("bb", [1, 256], BF16, kind="ExternalInput")
        ll = nc.dram_tensor("ll", [512, N_CLASSES], F32, kind="ExternalInput")
        wr = nc.dram_tensor("wr", [512, 1], F32, kind="ExternalInput")
        out = nc.dram_tensor("out", [BSH, N_CLASSES], F32, kind="ExternalOutput")
        with tile.TileContext(nc) as tc, ExitStack() as ctx:
            _emit(ctx, tc, xb.ap(), wT.ap(), bb.ap(), ll.ap(), wr.ap(), out.ap())
        nc.compile()
        _NC = nc
    return _NC


def _host_prep(x, tree_params, tree_weights):
    """Slice/layout the replicated params and cast x to bf16 (host-side)."""
    x = np.ascontiguousarray(np.asarray(x, np.float32)).astype(ml_dtypes.bfloat16)
    p = np.asarray(tree_params, np.float32)[0].reshape(N_TREES, PPT)
    w = p[:, :NW].reshape(N_TREES, N_INTERNAL, INPUT_DIM)[:, :4, :]
    wT = np.ascontiguousarray(
        w.reshape(N_TREES * 4, INPUT_DIM).T).astype(ml_dtypes.bfloat16)
    bias = np.ascontiguousarray(
        p[:, NW:NW + N_INTERNAL][:, :4].reshape(1, N_TREES * 4)
    ).astype(ml_dtypes.bfloat16)
    ll = np.ascontiguousarray(p[:, NW + N_INTERNAL:].reshape(512, N_CLASSES))
    wr = np.repeat(np.asarray(tree_weights, np.float32)[0], N_LEAVES)
    wr = np.ascontiguousarray(wr.reshape(512, 1))
    return x, wT, bias, ll, wr


def kernel(x: np.ndarray, tree_params: np.ndarray,
           tree_weights: np.ndarray) -> np.ndarray:
    from concourse.bass_utils import run_bass_kernel_spmd

    nc = _get_nc()
    xbf, wT, bias, ll, wr = _host_prep(x, tree_params, tree_weights)
    in_maps = [
        {"xb": xbf[c * BSH:(c + 1) * BSH], "wT": wT, "bb": bias,
         "ll": ll, "wr": wr}
        for c in range(N_CORES)
    ]
    res = run_bass_kernel_spmd(nc, in_maps, core_ids=list(range(N_CORES)))
    return np.concatenate([res.results[c]["out"] for c in range(N_CORES)], axis=0)


# revision 5
# speedup vs baseline: 2770.9137x; 1.7666x over previous
"""Data-parallel GeneratedTreeClassifier forward on 8 NeuronCores (Bass/Tile).

Shards the batch dim of x (16384 -> 8 x 2048) across cores, replicates the
small tree params, runs a hand-written Bass/Tile kernel per core, and
gathers the full [16384, 512] output.

Per-core device graph (batch tile = 128 rows, 16 tiles):
  xT   <- PE transpose (identity matmul) of bf16 x tile, ACT copy to SBUF
  z    = x @ W^T + bias      (PE, bf16, K=512+1)   -> PSUM [128, 256]
  d    = sigmoid(z)          (ACT)
  r    = 1 / (4 + d3 + eps)  (DVE approx)  [leaf sum is 4 + d3 algebraically]
  leaf = [r, d0*r, r-d0*r, d1*r, r-d1*r, d2*r, r-d2*r, d3*r]  (DVE/GPSIMD)
  lfT  <- PE transpose of leaf, DVE copy to SBUF
  out  = leaf @ M            (PE, bf16, K=512)     -> PSUM [128, 512]
  M    = softmax(leaf_logits) * tree_weight   (one-time, on device)
"""
import numpy as np
import ml_dtypes
from contextlib import ExitStack

import concourse.bass as bass
import concourse.tile as tile
from concourse import bacc, mybir
from concourse.masks import make_identity

INPUT_DIM = 512
N_CLASSES = 512
N_TREES = 64
N_LEAVES = 8
N_INTERNAL = 7
PPT = N_INTERNAL * (INPUT_DIM + 1) + N_LEAVES * N_CLASSES
BATCH = 16384
N_CORES = 8
BSH = BATCH // N_CORES          # 2048 rows per core
NB = BSH // 128                 # 16 batch tiles per core
NW = N_INTERNAL * INPUT_DIM
EPS = 1e-8

F32 = mybir.dt.float32
BF16 = mybir.dt.bfloat16


def _emit(ctx: ExitStack, tc, xb, wT, bb, ll, wr, out):
    nc = tc.nc

    const = ctx.enter_context(tc.tile_pool(name="const", bufs=1))

    ident = const.tile([128, 128], BF16)
    make_identity(nc, ident[:])

    # Replicated params, resident in SBUF.
    wt_sb = const.tile([128, 4, 256], BF16)
    nc.sync.dma_start(wt_sb[:], wT.rearrange("(k p) j -> p k j", p=128))
    bias_sb = const.tile([1, 256], BF16)
    nc.sync.dma_start(bias_sb[:], bb[:])
    ones_sb = const.tile([1, 128], BF16)
    nc.vector.memset(ones_sb[:], 1.0)
    m_sb = const.tile([128, 4, N_CLASSES], BF16)

    # M = softmax(leaf_logits, axis=-1) * w_tree   (rows tl = t*8 + l)
    ppool = ctx.enter_context(tc.tile_pool(name="prm", bufs=2))
    for k in range(4):
        llt = ppool.tile([128, N_CLASSES], F32, tag="llt")
        nc.sync.dma_start(llt[:], ll[k * 128:(k + 1) * 128, :])
        mx = ppool.tile([128, 1], F32, tag="mx")
        nc.vector.reduce_max(mx[:], llt[:], axis=mybir.AxisListType.X)
        nmx = ppool.tile([128, 1], F32, tag="nmx")
        nc.vector.tensor_scalar_mul(nmx[:], mx[:], -1.0)
        e = ppool.tile([128, N_CLASSES], F32, tag="e")
        s = ppool.tile([128, 1], F32, tag="s")
        nc.scalar.activation(e[:], llt[:], mybir.ActivationFunctionType.Exp,
                             bias=nmx[:], scale=1.0, accum_out=s[:])
        rs = ppool.tile([128, 1], F32, tag="rs")
        nc.vector.reciprocal(rs[:], s[:])
        wrt = ppool.tile([128, 1], F32, tag="wrt")
        nc.sync.dma_start(wrt[:], wr[k * 128:(k + 1) * 128, :])
        sc = ppool.tile([128, 1], F32, tag="sc")
        nc.vector.tensor_tensor(sc[:], rs[:], wrt[:], op=mybir.AluOpType.mult)
        nc.vector.tensor_scalar_mul(m_sb[:, k, :], e[:], sc[:])

    xpool = ctx.enter_context(tc.tile_pool(name="xin", bufs=3))
    spool = ctx.enter_context(tc.tile_pool(name="sbT", bufs=3))
    dpool = ctx.enter_context(tc.tile_pool(name="work", bufs=3))
    opool = ctx.enter_context(tc.tile_pool(name="osb", bufs=3))
    tpp = ctx.enter_context(tc.tile_pool(name="tps", bufs=3, space="PSUM"))
    zpp = ctx.enter_context(tc.tile_pool(name="zps", bufs=2, space="PSUM"))
    opp = ctx.enter_context(tc.tile_pool(name="ops", bufs=2, space="PSUM"))

    for i in range(NB):
        xin = xpool.tile([128, INPUT_DIM], BF16, tag="xin")
        nc.sync.dma_start(xin[:], xb[i * 128:(i + 1) * 128, :])

        # xT[p, k*128+b] = x[b, k*128+p]  via PE transpose
        tps = tpp.tile([128, 512], BF16, tag="tps")
        for k in range(4):
            nc.tensor.transpose(tps[:, k * 128:(k + 1) * 128],
                                xin[:, k * 128:(k + 1) * 128], ident[:])
        xT = spool.tile([128, 512], BF16, tag="xT")
        nc.scalar.copy(xT[:], tps[:])

        # z = x @ W^T + bias   -> PSUM [128b, 256j], j = t*4 + i
        zps = zpp.tile([128, 256], F32, tag="z")
        for k in range(4):
            nc.tensor.matmul(zps[:], lhsT=xT[:, k * 128:(k + 1) * 128],
                             rhs=wt_sb[:, k, :], start=(k == 0), stop=False)
        nc.tensor.matmul(zps[:], lhsT=ones_sb[:], rhs=bias_sb[:],
                         start=False, stop=True)

        d = dpool.tile([128, 256], F32, tag="d")
        nc.scalar.activation(d[:], zps[:], mybir.ActivationFunctionType.Sigmoid)
        dv = d[:].rearrange("p (t i) -> p t i", i=4)

        t4 = dpool.tile([128, 64], F32, tag="t4")
        nc.vector.tensor_scalar_add(t4[:], dv[:, :, 3], 4.0 + EPS)
        r = dpool.tile([128, 64], F32, tag="r")
        nc.vector.reciprocal_approx_fast(r[:], t4[:])

        # leaf[p, t, l] with l = 2q + m:
        #   l=0: r    l=1,3,5,7 (m=1): d_i*r    l=2,4,6 (q>=1, m=0): r-d_i*r
        leaf = dpool.tile([128, 64, 4, 2], BF16, tag="leaf")
        rb4 = r[:].unsqueeze(2).broadcast_to([128, 64, 4])
        rb3 = r[:].unsqueeze(2).broadcast_to([128, 64, 3])
        nc.gpsimd.tensor_copy(leaf[:, :, 0, 0], r[:])
        nc.vector.tensor_tensor(leaf[:, :, :, 1], dv[:], rb4,
                                op=mybir.AluOpType.mult)
        nc.gpsimd.tensor_tensor(leaf[:, :, 1:4, 0], rb3, leaf[:, :, 0:3, 1],
                                op=mybir.AluOpType.subtract)

        # lfT[p, k*128+b] = leaf_flat[b, k*128+p]  via PE transpose
        lv = leaf[:].rearrange("p t q m -> p (t q m)")
        tps2 = tpp.tile([128, 512], BF16, tag="tps")
        for k in range(4):
            nc.tensor.transpose(tps2[:, k * 128:(k + 1) * 128],
                                lv[:, k * 128:(k + 1) * 128], ident[:])
        lfT = spool.tile([128, 512], BF16, tag="lfT")
        nc.vector.tensor_copy(lfT[:], tps2[:])

        # out = leaf @ M  -> PSUM [128b, 512c]
        ops = opp.tile([128, 512], F32, tag="o")
        for k in range(4):
            nc.tensor.matmul(ops[:], lhsT=lfT[:, k * 128:(k + 1) * 128],
                             rhs=m_sb[:, k, :], start=(k == 0), stop=(k == 3))
        osb = opool.tile([128, 512], F32, tag="osb")
        nc.scalar.copy(osb[:], ops[:])
        nc.sync.dma_start(out[i * 128:(i + 1) * 128, :], osb[:])


_NC = None
_RUNNER = None


def _get_nc():
    global _NC
    if _NC is None:
        nc = bacc.Bacc("TRN2", target_bir_lowering=False, debug=False)
        xb = nc.dram_tensor("xb", [BSH, INPUT_DIM], BF16, kind="ExternalInput")
        wT = nc.dram_tensor("wT", [INPUT_DIM, 256], BF16, kind="ExternalInput")
        bb = nc.dram_tensor("bb", [1, 256], BF16, kind="ExternalInput")
        ll = nc.dram_tensor("ll", [512, N_CLASSES], F32, kind="ExternalInput")
        wr = nc.dram_tensor("wr", [512, 1], F32, kind="ExternalInput")
        out = nc.dram_tensor("out", [BSH, N_CLASSES], F32, kind="ExternalOutput")
        with tile.TileContext(nc) as tc, ExitStack() as ctx:
            _emit(ctx, tc, xb.ap(), wT.ap(), bb.ap(), ll.ap(), wr.ap(), out.ap())
        nc.compile()
        _NC = nc
    return _NC


def _get_runner():
    """Build the sharded PJRT executable ONCE (jit + NEFF compile are cached
    across kernel() calls; run_bass_kernel_spmd would re-trace every call)."""
    global _RUNNER
    if _RUNNER is None:
        import jax
        import jax.numpy as jnp
        from jax.sharding import Mesh, PartitionSpec, NamedSharding
        from jax.experimental.shard_map import shard_map
        from concourse import bass2jax

        nc = _get_nc()
        bass2jax.install_neuronx_cc_hook()

        part_name = (nc.partition_id_tensor.name
                     if nc.partition_id_tensor else None)
        in_names, out_names, out_avals = [], [], []
        for alloc in nc.m.functions[0].allocations:
            if not isinstance(alloc, mybir.MemoryLocationSet):
                continue
            name = alloc.memorylocations[0].name
            if alloc.kind == "ExternalInput":
                if name != part_name:
                    in_names.append(name)
            elif alloc.kind == "ExternalOutput":
                out_names.append(name)
                out_avals.append(jax.core.ShapedArray(
                    tuple(alloc.tensor_shape), mybir.dt.np(alloc.dtype)))
        n_params = len(in_names)
        all_names = tuple(in_names) + tuple(out_names)
        if part_name is not None:
            all_names = all_names + (part_name,)
        donate = tuple(range(n_params, n_params + len(out_names)))

        def _body(*args):
            operands = list(args)
            if part_name is not None:
                operands.append(bass2jax.partition_id_tensor())
            outs = bass2jax._bass_exec_p.bind(
                *operands,
                out_avals=tuple(out_avals),
                in_names=all_names,
                out_names=tuple(out_names),
                lowering_input_output_aliases=(),
                sim_require_finite=True,
                sim_require_nnan=True,
                nc=nc,
            )
            return tuple(outs)

        devices = jax.devices()[:N_CORES]
        mesh = Mesh(np.asarray(devices), ("core",))
        spec = PartitionSpec("core")
        fn = jax.jit(
            shard_map(_body, mesh=mesh,
                      in_specs=(spec,) * (n_params + len(out_names)),
                      out_specs=(spec,) * len(out_names), check_rep=False),
            donate_argnums=donate, keep_unused=True)
        zmk = jax.jit(
            lambda: jnp.zeros((N_CORES * BSH, N_CLASSES), jnp.float32),
            out_shardings=NamedSharding(mesh, spec))
        _RUNNER = (fn, zmk, in_names)
    return _RUNNER


def _host_prep(x, tree_params, tree_weights):
    """Slice/layout the replicated params and cast x to bf16 (host-side)."""
    x = np.ascontiguousarray(np.asarray(x, np.float32)).astype(ml_dtypes.bfloat16)
    p = np.asarray(tree_params, np.float32)[0].reshape(N_TREES, PPT)
    w = p[:, :NW].reshape(N_TREES, N_INTERNAL, INPUT_DIM)[:, :4, :]
    wT = np.ascontiguousarray(
        w.reshape(N_TREES * 4, INPUT_DIM).T).astype(ml_dtypes.bfloat16)
    bias = np.ascontiguousarray(
        p[:, NW:NW + N_INTERNAL][:, :4].reshape(1, N_TREES * 4)
    ).astype(ml_dtypes.bfloat16)
    ll = np.ascontiguousarray(p[:, NW + N_INTERNAL:].reshape(512, N_CLASSES))
    wr = np.repeat(np.asarray(tree_weights, np.float32)[0], N_LEAVES)
    wr = np.ascontiguousarray(wr.reshape(512, 1))
    return x, wT, bias, ll, wr


def kernel(x: np.ndarray, tree_params: np.ndarray,
           tree_weights: np.ndarray) -> np.ndarray:
    fn, zmk, in_names = _get_runner()
    xbf, wT, bias, ll, wr = _host_prep(x, tree_params, tree_weights)
    reps = {"xb": xbf,
            "wT": np.concatenate([wT] * N_CORES, 0),
            "bb": np.concatenate([bias] * N_CORES, 0),
            "ll": np.concatenate([ll] * N_CORES, 0),
            "wr": np.concatenate([wr] * N_CORES, 0)}
    args = [reps[n] for n in in_names] + [zmk()]
    outs = fn(*args)
    return np.asarray(outs[0])


# revision 12
# speedup vs baseline: 4693.5289x; 1.6939x over previous
"""Data-parallel GeneratedTreeClassifier forward on 8 NeuronCores (Bass/Tile).

Shards the batch dim of x (16384 -> 8 x 2048) across cores, replicates the
small tree params, runs a hand-written Bass/Tile kernel per core, and
gathers the full [16384, 512] output.

Math restructure (per tree t, decision i = 0..3, r = 1/(4 + d3 + eps)):
  out = leaf_norm @ (softmax(leaf_logits) * w)
      = r @ C + (r*d) @ G
  C_t = M_t0 + M_t2 + M_t4 + M_t6
  G_(t,i) = M_t(1+2i) - M_t(2+2i)  (i<3),   G_(t,3) = M_t7
which kills the per-tile leaf assembly + transpose and shrinks mm2's K
from 512 to 320.

Per-core device graph, processed in groups of 4 batch tiles (512 rows):
  xT   <- PE transpose (identity matmul) of bf16 x tiles  [128d, 4k, 512b]
  zT   = W @ x^T            (PE, j-major: 8 matmuls of N=512)
  d    = sigmoid(zT + bias) (ACT, bias per-partition)     [128j, 2, 512b]
  r    = 1/(4+d3+eps)       (DVE approx, partitions 64:128)
  e    = d * r              (DVE, bf16)   -> mm2 lhsT tiles T0, T1
  out  = [e; r] @ [G; C]    (PE, 3 matmuls of N=512 per batch tile)
  M    = softmax(leaf_logits)*w, C/G via pattern matmuls (one-time).
"""
import numpy as np
import ml_dtypes
from contextlib import ExitStack

import concourse.bass as bass
import concourse.tile as tile
from concourse import bacc, mybir
from concourse.masks import make_identity

INPUT_DIM = 512
N_CLASSES = 512
N_TREES = 64
N_LEAVES = 8
N_INTERNAL = 7
PPT = N_INTERNAL * (INPUT_DIM + 1) + N_LEAVES * N_CLASSES
BATCH = 16384
N_CORES = 8
BSH = BATCH // N_CORES          # 2048 rows per core
NB = BSH // 128                 # 16 batch tiles per core
NG = NB // 4                    # 4 groups of 4 tiles
NW = N_INTERNAL * INPUT_DIM
EPS = 1e-8

F32 = mybir.dt.float32
BF16 = mybir.dt.bfloat16


def _emit(ctx: ExitStack, tc, xb, wT, bb, bb2, ll, wr, pc, pg, out):
    nc = tc.nc
    AF = mybir.ActivationFunctionType

    const = ctx.enter_context(tc.tile_pool(name="const", bufs=1))

    ident = const.tile([128, 128], BF16)
    make_identity(nc, ident[:])

    # Replicated params, resident in SBUF.
    wt_sb = const.tile([128, 4, 256], BF16)          # [d%128, dk, j]
    nc.sync.dma_start(wt_sb[:], wT.rearrange("(k p) j -> p k j", p=128))
    bias_sb = const.tile([128, 2], F32)              # [j%128, jb]
    nc.sync.dma_start(bias_sb[:], bb.rearrange("(jb p) one -> p (jb one)", p=128))
    bias2_sb = const.tile([128, 1], F32)             # b3 + ln(1.25) at 64:128
    nc.sync.dma_start(bias2_sb[64:128, :], bb2[:])
    pc_sb = const.tile([128, 4, 64], BF16)           # [tl%128, tlk, t]
    nc.sync.dma_start(pc_sb[:], pc.rearrange("(k p) t -> p k t", p=128))
    pg_sb = const.tile([128, 4, 256], BF16)          # [tl%128, tlk, j]
    nc.sync.dma_start(pg_sb[:], pg.rearrange("(k p) j -> p k j", p=128))
    m_sb = const.tile([128, 4, N_CLASSES], BF16)     # [tl%128, tlk, c]
    cg0 = const.tile([128, N_CLASSES], BF16)         # G rows (i0; i1)
    cg1 = const.tile([128, N_CLASSES], BF16)         # G rows (i2; i3)
    cg2 = const.tile([128, N_CLASSES], BF16)         # C rows at 64:128

    # M = softmax(leaf_logits, axis=-1) * w_tree   (rows tl = t*8 + l)
    ppool = ctx.enter_context(tc.tile_pool(name="prm", bufs=2))
    for k in range(4):
        llt = ppool.tile([128, N_CLASSES], F32, tag="llt")
        nc.sync.dma_start(llt[:], ll[k * 128:(k + 1) * 128, :])
        mx = ppool.tile([128, 1], F32, tag="mx")
        nc.vector.reduce_max(mx[:], llt[:], axis=mybir.AxisListType.X)
        nmx = ppool.tile([128, 1], F32, tag="nmx")
        nc.vector.tensor_scalar_mul(nmx[:], mx[:], -1.0)
        e = ppool.tile([128, N_CLASSES], F32, tag="e")
        s = ppool.tile([128, 1], F32, tag="s")
        nc.scalar.activation(e[:], llt[:], AF.Exp,
                             bias=nmx[:], scale=1.0, accum_out=s[:])
        rs = ppool.tile([128, 1], F32, tag="rs")
        nc.vector.reciprocal(rs[:], s[:])
        wrt = ppool.tile([128, 1], F32, tag="wrt")
        nc.sync.dma_start(wrt[:], wr[k * 128:(k + 1) * 128, :])
        sc = ppool.tile([128, 1], F32, tag="sc")
        nc.vector.tensor_tensor(sc[:], rs[:], wrt[:], op=mybir.AluOpType.mult)
        nc.vector.tensor_scalar_mul(m_sb[:, k, :], e[:], sc[:])

    xpool = ctx.enter_context(tc.tile_pool(name="xin", bufs=6))
    spool = ctx.enter_context(tc.tile_pool(name="xT4", bufs=3))
    dpool = ctx.enter_context(tc.tile_pool(name="work", bufs=3))
    epool = ctx.enter_context(tc.tile_pool(name="eT", bufs=3))
    opool = ctx.enter_context(tc.tile_pool(name="osb", bufs=6))
    tpp = ctx.enter_context(tc.tile_pool(name="tps", bufs=2, space="PSUM"))
    zpp = ctx.enter_context(tc.tile_pool(name="zps", bufs=2, space="PSUM"))
    opp = ctx.enter_context(tc.tile_pool(name="ops", bufs=2, space="PSUM"))

    # One-time: C/G from M via host-provided 0/±1 pattern matrices.
    cg2ps = opp.tile([128, 512], F32, tag="o")
    for k in range(4):
        nc.tensor.matmul(cg2ps[64:128, :], lhsT=pc_sb[:, k, :],
                         rhs=m_sb[:, k, :], start=(k == 0), stop=(k == 3))
    nc.scalar.copy(cg2[64:128, :], cg2ps[64:128, :])
    cg0ps = opp.tile([128, 512], F32, tag="o")
    for k in range(4):
        nc.tensor.matmul(cg0ps[:], lhsT=pg_sb[:, k, 0:128],
                         rhs=m_sb[:, k, :], start=(k == 0), stop=(k == 3))
    nc.scalar.copy(cg0[:], cg0ps[:])
    cg1ps = opp.tile([128, 512], F32, tag="o")
    for k in range(4):
        nc.tensor.matmul(cg1ps[:], lhsT=pg_sb[:, k, 128:256],
                         rhs=m_sb[:, k, :], start=(k == 0), stop=(k == 3))
    nc.scalar.copy(cg1[:], cg1ps[:])

    for g in range(NG):
        # x tiles transposed: xT4[p, k, bt*128+b] = x[(4g+bt)*128+b, k*128+p]
        xT4 = spool.tile([128, 4, 512], BF16, tag="xT4")
        for bt in range(4):
            xin = xpool.tile([128, INPUT_DIM], BF16, tag="xin")
            nc.sync.dma_start(xin[:], xb[(4 * g + bt) * 128:
                                         (4 * g + bt + 1) * 128, :])
            tps = tpp.tile([128, 512], BF16, tag="tps")
            for k in range(4):
                nc.tensor.transpose(tps[:, k * 128:(k + 1) * 128],
                                    xin[:, k * 128:(k + 1) * 128], ident[:])
            nc.scalar.copy(xT4[:, :, bt * 128:(bt + 1) * 128],
                           tps[:].rearrange("p (k b) -> p k b", k=4))

        # zT[j, b] = sum_d W[j, d] x[b, d]    j = i*64 + t, i-major
        zt = zpp.tile([128, 2, 512], F32, tag="zt")
        for jb in range(2):
            for k in range(4):
                nc.tensor.matmul(zt[:, jb, :],
                                 lhsT=wt_sb[:, k, jb * 128:(jb + 1) * 128],
                                 rhs=xT4[:, k, :],
                                 start=(k == 0), stop=(k == 3))
        d4 = dpool.tile([128, 2, 512], BF16, tag="d4")
        for jb in range(2):
            nc.scalar.activation(d4[:, jb, :], zt[:, jb, :], AF.Sigmoid,
                                 bias=bias_sb[:, jb:jb + 1])

        # r = 1/(4 + d3) = 1/4 - sigmoid(z3 + ln 1.25)/20   (exact identity)
        s3 = dpool.tile([128, 512], F32, tag="s3")
        nc.scalar.activation(s3[64:128, :], zt[64:128, 1, :], AF.Sigmoid,
                             bias=bias2_sb[64:128, :])
        rb = dpool.tile([128, 512], BF16, tag="rb")
        nc.vector.tensor_scalar(rb[64:128, :], s3[64:128, :], -0.05, 0.25,
                                op0=mybir.AluOpType.mult,
                                op1=mybir.AluOpType.add)
        # replicate r to all (i, jb) lanes:  r4[a*64+t, jb, b] = r[t, b]
        r4 = dpool.tile([128, 2, 512], BF16, tag="r4")
        for jb in range(2):
            for a in range(2):
                nc.sync.dma_start(r4[a * 64:(a + 1) * 64, jb, :],
                                  rb[64:128, :])

        # e = d * r  -> lhsT tiles for mm2 (rows i*64+t match G rows)
        T0 = epool.tile([128, 512], BF16, tag="T0")
        T1 = epool.tile([128, 512], BF16, tag="T1")
        nc.vector.tensor_tensor(T0[:], d4[:, 0, :], r4[:, 0, :],
                                op=mybir.AluOpType.mult)
        nc.vector.tensor_tensor(T1[:], d4[:, 1, :], r4[:, 1, :],
                                op=mybir.AluOpType.mult)

        # out = e @ G + r @ C  per batch tile
        for bt in range(4):
            bs = slice(bt * 128, (bt + 1) * 128)
            ops = opp.tile([128, 512], F32, tag="o")
            nc.tensor.matmul(ops[:], lhsT=T0[:, bs], rhs=cg0[:],
                             start=True, stop=False)
            nc.tensor.matmul(ops[:], lhsT=T1[:, bs], rhs=cg1[:],
                             start=False, stop=False)
            nc.tensor.matmul(ops[:], lhsT=rb[64:128, bs], rhs=cg2[64:128, :],
                             start=False, stop=True)
            osb = opool.tile([128, 512], BF16, tag="osb")
            nc.scalar.copy(osb[:], ops[:])
            nc.sync.dma_start(out[(4 * g + bt) * 128:(4 * g + bt + 1) * 128, :],
                              osb[:])


_NC = None
_RUNNER = None


def _get_nc():
    global _NC
    if _NC is None:
        nc = bacc.Bacc("TRN2", target_bir_lowering=False, debug=False)
        xb = nc.dram_tensor("xb", [BSH, INPUT_DIM], BF16, kind="ExternalInput")
        wT = nc.dram_tensor("wT", [INPUT_DIM, 256], BF16, kind="ExternalInput")
        bb = nc.dram_tensor("bb", [256, 1], F32, kind="ExternalInput")
        bb2 = nc.dram_tensor("bb2", [64, 1], F32, kind="ExternalInput")
        ll = nc.dram_tensor("ll", [512, N_CLASSES], F32, kind="ExternalInput")
        wr = nc.dram_tensor("wr", [512, 1], F32, kind="ExternalInput")
        pc = nc.dram_tensor("pc", [512, 64], BF16, kind="ExternalInput")
        pg = nc.dram_tensor("pg", [512, 256], BF16, kind="ExternalInput")
        out = nc.dram_tensor("out", [BSH, N_CLASSES], BF16, kind="ExternalOutput")
        with tile.TileContext(nc) as tc, ExitStack() as ctx:
            _emit(ctx, tc, xb.ap(), wT.ap(), bb.ap(), bb2.ap(), ll.ap(), wr.ap(),
                  pc.ap(), pg.ap(), out.ap())
        nc.compile()
        _NC = nc
    return _NC


def _get_runner():
    """Build the sharded PJRT executable ONCE (jit + NEFF compile are cached
    across kernel() calls; run_bass_kernel_spmd would re-trace every call)."""
    global _RUNNER
    if _RUNNER is None:
        import jax
        import jax.numpy as jnp
        from jax.sharding import Mesh, PartitionSpec, NamedSharding
        from jax.experimental.shard_map import shard_map
        from concourse import bass2jax

        nc = _get_nc()
        bass2jax.install_neuronx_cc_hook()

        part_name = (nc.partition_id_tensor.name
                     if nc.partition_id_tensor else None)
        in_names, out_names, out_avals = [], [], []
        for alloc in nc.m.functions[0].allocations:
            if not isinstance(alloc, mybir.MemoryLocationSet):
                continue
            name = alloc.memorylocations[0].name
            if alloc.kind == "ExternalInput":
                if name != part_name:
                    in_names.append(name)
            elif alloc.kind == "ExternalOutput":
                out_names.append(name)
                out_avals.append(jax.core.ShapedArray(
                    tuple(alloc.tensor_shape), mybir.dt.np(alloc.dtype)))
        n_params = len(in_names)
        all_names = tuple(in_names) + tuple(out_names)
        if part_name is not None:
            all_names = all_names + (part_name,)
        donate = tuple(range(n_params, n_params + len(out_names)))

        def _body(*args):
            operands = list(args)
            if part_name is not None:
                operands.append(bass2jax.partition_id_tensor())
            outs = bass2jax._bass_exec_p.bind(
                *operands,
                out_avals=tuple(out_avals),
                in_names=all_names,
                out_names=tuple(out_names),
                lowering_input_output_aliases=(),
                sim_require_finite=True,
                sim_require_nnan=True,
                nc=nc,
            )
            return tuple(outs)

        devices = jax.devices()[:N_CORES]
        mesh = Mesh(np.asarray(devices), ("core",))
        spec = PartitionSpec("core")
        fn = jax.jit(
            shard_map(_body, mesh=mesh,
                      in_specs=(spec,) * (n_params + len(out_names)),
                      out_specs=(spec,) * len(out_names), check_rep=False),
            donate_argnums=donate, keep_unused=True)
        zmk = jax.jit(
            lambda: jnp.zeros((N_CORES * BSH, N_CLASSES), ml_dtypes.bfloat16),
            out_shardings=NamedSharding(mesh, spec))
        _RUNNER = (fn, zmk, in_names)
    return _RUNNER


def _patterns():
    """0/±1 combination matrices: C = PC^T M, G = PG^T M (tl = 8t + l)."""
    pcm = np.zeros((512, 64), np.float32)
    pgm = np.zeros((512, 256), np.float32)
    for t in range(N_TREES):
        for l in (0, 2, 4, 6):
            pcm[8 * t + l, t] = 1.0
        for i in range(3):
            pgm[8 * t + 1 + 2 * i, i * 64 + t] = 1.0
            pgm[8 * t + 2 + 2 * i, i * 64 + t] = -1.0
        pgm[8 * t + 7, 3 * 64 + t] = 1.0
    return (pcm.astype(ml_dtypes.bfloat16), pgm.astype(ml_dtypes.bfloat16))


_PC, _PG = _patterns()


def _host_prep(x, tree_params, tree_weights):
    """Slice/layout the replicated params and cast x to bf16 (host-side)."""
    x = np.ascontiguousarray(np.asarray(x, np.float32)).astype(ml_dtypes.bfloat16)
    p = np.asarray(tree_params, np.float32)[0].reshape(N_TREES, PPT)
    w = p[:, :NW].reshape(N_TREES, N_INTERNAL, INPUT_DIM)[:, :4, :]
    # j = i*64 + t (i-major)
    w_im = np.ascontiguousarray(w.transpose(1, 0, 2).reshape(256, INPUT_DIM))
    wT = np.ascontiguousarray(w_im.T).astype(ml_dtypes.bfloat16)
    bias = np.ascontiguousarray(
        p[:, NW:NW + N_INTERNAL][:, :4].T.reshape(256, 1))
    bias2 = np.ascontiguousarray(bias[192:256] + np.float32(np.log(1.25)))
    ll = np.ascontiguousarray(p[:, NW + N_INTERNAL:].reshape(512, N_CLASSES))
    wr = np.repeat(np.asarray(tree_weights, np.float32)[0], N_LEAVES)
    wr = np.ascontiguousarray(wr.reshape(512, 1))
    return x, wT, bias, bias2, ll, wr


def kernel(x: np.ndarray, tree_params: np.ndarray,
           tree_weights: np.ndarray) -> np.ndarray:
    fn, zmk, in_names = _get_runner()
    xbf, wT, bias, bias2, ll, wr = _host_prep(x, tree_params, tree_weights)
    reps = {"xb": xbf,
            "wT": np.concatenate([wT] * N_CORES, 0),
            "bb": np.concatenate([bias] * N_CORES, 0),
            "bb2": np.concatenate([bias2] * N_CORES, 0),
            "ll": np.concatenate([ll] * N_CORES, 0),
            "wr": np.concatenate([wr] * N_CORES, 0),
            "pc": np.concatenate([_PC] * N_CORES, 0),
            "pg": np.concatenate([_PG] * N_CORES, 0)}
    args = [reps[n] for n in in_names] + [zmk()]
    outs = fn(*args)
    return np.asarray(outs[0]).astype(np.float32)


# revision 13
# speedup vs baseline: 5473.2122x; 1.1661x over previous
"""Data-parallel GeneratedTreeClassifier forward on 8 NeuronCores (Bass/Tile).

Shards the batch dim of x (16384 -> 8 x 2048) across cores, replicates the
small tree params, runs a hand-written Bass/Tile kernel per core, and
gathers the full [16384, 512] output.

Math restructure (per tree t, decision i = 0..3, r = 1/(4 + d3 + eps)):
  out = leaf_norm @ (softmax(leaf_logits) * w)
      = r @ C + (r*d) @ G
  C_t = M_t0 + M_t2 + M_t4 + M_t6
  G_(t,i) = M_t(1+2i) - M_t(2+2i)  (i<3),   G_(t,3) = M_t7
which kills the per-tile leaf assembly + transpose and shrinks mm2's K
from 512 to 320.

Per-core device graph, processed in groups of 4 batch tiles (512 rows):
  xT   <- PE transpose (identity matmul) of bf16 x tiles  [128d, 4k, 512b]
  zT   = W @ x^T            (PE, j-major: 8 matmuls of N=512)
  d    = sigmoid(zT + bias) (ACT, bias per-partition)     [128j, 2, 512b]
  r    = 1/(4+d3+eps)       (DVE approx, partitions 64:128)
  e    = d * r              (DVE, bf16)   -> mm2 lhsT tiles T0, T1
  out  = [e; r] @ [G; C]    (PE, 3 matmuls of N=512 per batch tile)
  M    = softmax(leaf_logits)*w, C/G via pattern matmuls (one-time).
"""
import numpy as np
import ml_dtypes
from contextlib import ExitStack

import concourse.bass as bass
import concourse.tile as tile
from concourse import bacc, mybir

INPUT_DIM = 512
N_CLASSES = 512
N_TREES = 64
N_LEAVES = 8
N_INTERNAL = 7
PPT = N_INTERNAL * (INPUT_DIM + 1) + N_LEAVES * N_CLASSES
BATCH = 16384
N_CORES = 8
BSH = BATCH // N_CORES          # 2048 rows per core
NB = BSH // 128                 # 16 batch tiles per core
NG = NB // 4                    # 4 groups of 4 tiles
NW = N_INTERNAL * INPUT_DIM
EPS = 1e-8

F32 = mybir.dt.float32
BF16 = mybir.dt.bfloat16


def _emit(ctx: ExitStack, tc, xt, wT, bb, bb2, ll, wr, pc, pg, out):
    nc = tc.nc
    AF = mybir.ActivationFunctionType

    const = ctx.enter_context(tc.tile_pool(name="const", bufs=1))

    # Replicated params, resident in SBUF.
    wt_sb = const.tile([128, 4, 256], BF16)          # [d%128, dk, j]
    nc.sync.dma_start(wt_sb[:], wT.rearrange("(k p) j -> p k j", p=128))
    bias_sb = const.tile([128, 2], F32)              # [j%128, jb]
    nc.sync.dma_start(bias_sb[:], bb.rearrange("(jb p) one -> p (jb one)", p=128))
    bias2_sb = const.tile([128, 1], F32)             # b3 + ln(1.25) at 64:128
    nc.sync.dma_start(bias2_sb[64:128, :], bb2[:])
    pc_sb = const.tile([128, 4, 64], BF16)           # [tl%128, tlk, t]
    nc.sync.dma_start(pc_sb[:], pc.rearrange("(k p) t -> p k t", p=128))
    pg_sb = const.tile([128, 4, 256], BF16)          # [tl%128, tlk, j]
    nc.sync.dma_start(pg_sb[:], pg.rearrange("(k p) j -> p k j", p=128))
    m_sb = const.tile([128, 4, N_CLASSES], BF16)     # [tl%128, tlk, c]
    cg0 = const.tile([128, N_CLASSES], BF16)         # G rows (i0; i1)
    cg1 = const.tile([128, N_CLASSES], BF16)         # G rows (i2; i3)
    cg2 = const.tile([128, N_CLASSES], BF16)         # C rows at 64:128

    # M = softmax(leaf_logits, axis=-1) * w_tree   (rows tl = t*8 + l)
    ppool = ctx.enter_context(tc.tile_pool(name="prm", bufs=2))
    for k in range(4):
        llt = ppool.tile([128, N_CLASSES], F32, tag="llt")
        nc.sync.dma_start(llt[:], ll[k * 128:(k + 1) * 128, :])
        mx = ppool.tile([128, 1], F32, tag="mx")
        nc.vector.reduce_max(mx[:], llt[:], axis=mybir.AxisListType.X)
        nmx = ppool.tile([128, 1], F32, tag="nmx")
        nc.vector.tensor_scalar_mul(nmx[:], mx[:], -1.0)
        e = ppool.tile([128, N_CLASSES], F32, tag="e")
        s = ppool.tile([128, 1], F32, tag="s")
        nc.scalar.activation(e[:], llt[:], AF.Exp,
                             bias=nmx[:], scale=1.0, accum_out=s[:])
        rs = ppool.tile([128, 1], F32, tag="rs")
        nc.vector.reciprocal(rs[:], s[:])
        wrt = ppool.tile([128, 1], F32, tag="wrt")
        nc.sync.dma_start(wrt[:], wr[k * 128:(k + 1) * 128, :])
        sc = ppool.tile([128, 1], F32, tag="sc")
        nc.vector.tensor_tensor(sc[:], rs[:], wrt[:], op=mybir.AluOpType.mult)
        nc.vector.tensor_scalar_mul(m_sb[:, k, :], e[:], sc[:])

    spool = ctx.enter_context(tc.tile_pool(name="xT", bufs=1))
    dpool = ctx.enter_context(tc.tile_pool(name="work", bufs=3))
    epool = ctx.enter_context(tc.tile_pool(name="eT", bufs=3))
    opool = ctx.enter_context(tc.tile_pool(name="osb", bufs=6))
    zpp = ctx.enter_context(tc.tile_pool(name="zps", bufs=3, space="PSUM"))
    opp = ctx.enter_context(tc.tile_pool(name="ops", bufs=2, space="PSUM"))

    # One-time: C/G from M via host-provided 0/±1 pattern matrices.
    cg2ps = opp.tile([128, 512], F32, tag="o")
    for k in range(4):
        nc.tensor.matmul(cg2ps[64:128, :], lhsT=pc_sb[:, k, :],
                         rhs=m_sb[:, k, :], start=(k == 0), stop=(k == 3))
    nc.scalar.copy(cg2[64:128, :], cg2ps[64:128, :])
    cg0ps = opp.tile([128, 512], F32, tag="o")
    for k in range(4):
        nc.tensor.matmul(cg0ps[:], lhsT=pg_sb[:, k, 0:128],
                         rhs=m_sb[:, k, :], start=(k == 0), stop=(k == 3))
    nc.scalar.copy(cg0[:], cg0ps[:])
    cg1ps = opp.tile([128, 512], F32, tag="o")
    for k in range(4):
        nc.tensor.matmul(cg1ps[:], lhsT=pg_sb[:, k, 128:256],
                         rhs=m_sb[:, k, :], start=(k == 0), stop=(k == 3))
    nc.scalar.copy(cg1[:], cg1ps[:])

    # x^T resident in SBUF (pre-transposed on host): xT[p, k, b] = x[b, k*128+p]
    xT = spool.tile([128, 4, BSH], BF16)
    for k in range(4):
        nc.sync.dma_start(xT[:, k, :], xt[k * 128:(k + 1) * 128, :])

    for g in range(NG):
        # zT[j, b] = sum_d W[j, d] x[b, d]    j = i*64 + t, i-major
        gs = slice(g * 512, (g + 1) * 512)
        zt = zpp.tile([128, 2, 512], F32, tag="zt")
        for jb in range(2):
            for k in range(4):
                nc.tensor.matmul(zt[:, jb, :],
                                 lhsT=wt_sb[:, k, jb * 128:(jb + 1) * 128],
                                 rhs=xT[:, k, gs],
                                 start=(k == 0), stop=(k == 3))
        d4 = dpool.tile([128, 2, 512], BF16, tag="d4")
        for jb in range(2):
            nc.scalar.activation(d4[:, jb, :], zt[:, jb, :], AF.Sigmoid,
                                 bias=bias_sb[:, jb:jb + 1])

        # r = 1/(4 + d3) = 1/4 - sigmoid(z3 + ln 1.25)/20   (exact identity)
        s3 = dpool.tile([128, 512], F32, tag="s3")
        nc.scalar.activation(s3[64:128, :], zt[64:128, 1, :], AF.Sigmoid,
                             bias=bias2_sb[64:128, :])
        rb = dpool.tile([128, 512], BF16, tag="rb")
        nc.vector.tensor_scalar(rb[64:128, :], s3[64:128, :], -0.05, 0.25,
                                op0=mybir.AluOpType.mult,
                                op1=mybir.AluOpType.add)
        # replicate r to all (i, jb) lanes:  r4[a*64+t, jb, b] = r[t, b]
        r4 = dpool.tile([128, 2, 512], BF16, tag="r4")
        for jb in range(2):
            for a in range(2):
                nc.sync.dma_start(r4[a * 64:(a + 1) * 64, jb, :],
                                  rb[64:128, :])

        # e = d * r  -> lhsT tiles for mm2 (rows i*64+t match G rows)
        T0 = epool.tile([128, 512], BF16, tag="T0")
        T1 = epool.tile([128, 512], BF16, tag="T1")
        nc.vector.tensor_tensor(T0[:], d4[:, 0, :], r4[:, 0, :],
                                op=mybir.AluOpType.mult)
        nc.vector.tensor_tensor(T1[:], d4[:, 1, :], r4[:, 1, :],
                                op=mybir.AluOpType.mult)

        # out = e @ G + r @ C  per batch tile
        for bt in range(4):
            bs = slice(bt * 128, (bt + 1) * 128)
            ops = opp.tile([128, 512], F32, tag="o")
            nc.tensor.matmul(ops[:], lhsT=T0[:, bs], rhs=cg0[:],
                             start=True, stop=False)
            nc.tensor.matmul(ops[:], lhsT=T1[:, bs], rhs=cg1[:],
                             start=False, stop=False)
            nc.tensor.matmul(ops[:], lhsT=rb[64:128, bs], rhs=cg2[64:128, :],
                             start=False, stop=True)
            osb = opool.tile([128, 512], BF16, tag="osb")
            nc.scalar.copy(osb[:], ops[:])
            nc.sync.dma_start(out[(4 * g + bt) * 128:(4 * g + bt + 1) * 128, :],
                              osb[:])


_NC = None
_RUNNER = None


def _get_nc():
    global _NC
    if _NC is None:
        nc = bacc.Bacc("TRN2", target_bir_lowering=False, debug=False)
        xt = nc.dram_tensor("xt", [INPUT_DIM, BSH], BF16, kind="ExternalInput")
        wT = nc.dram_tensor("wT", [INPUT_DIM, 256], BF16, kind="ExternalInput")
        bb = nc.dram_tensor("bb", [256, 1], F32, kind="ExternalInput")
        bb2 = nc.dram_tensor("bb2", [64, 1], F32, kind="ExternalInput")
        ll = nc.dram_tensor("ll", [512, N_CLASSES], F32, kind="ExternalInput")
        wr = nc.dram_tensor("wr", [512, 1], F32, kind="ExternalInput")
        pc = nc.dram_tensor("pc", [512, 64], BF16, kind="ExternalInput")
        pg = nc.dram_tensor("pg", [512, 256], BF16, kind="ExternalInput")
        out = nc.dram_tensor("out", [BSH, N_CLASSES], BF16, kind="ExternalOutput")
        with tile.TileContext(nc) as tc, ExitStack() as ctx:
            _emit(ctx, tc, xt.ap(), wT.ap(), bb.ap(), bb2.ap(), ll.ap(), wr.ap(),
                  pc.ap(), pg.ap(), out.ap())
        nc.compile()
        _NC = nc
    return _NC


def _get_runner():
    """Build the sharded PJRT executable ONCE (jit + NEFF compile are cached
    across kernel() calls; run_bass_kernel_spmd would re-trace every call)."""
    global _RUNNER
    if _RUNNER is None:
        import jax
        import jax.numpy as jnp
        from jax.sharding import Mesh, PartitionSpec, NamedSharding
        from jax.experimental.shard_map import shard_map
        from concourse import bass2jax

        nc = _get_nc()
        bass2jax.install_neuronx_cc_hook()

        part_name = (nc.partition_id_tensor.name
                     if nc.partition_id_tensor else None)
        in_names, out_names, out_avals = [], [], []
        for alloc in nc.m.functions[0].allocations:
            if not isinstance(alloc, mybir.MemoryLocationSet):
                continue
            name = alloc.memorylocations[0].name
            if alloc.kind == "ExternalInput":
                if name != part_name:
                    in_names.append(name)
            elif alloc.kind == "ExternalOutput":
                out_names.append(name)
                out_avals.append(jax.core.ShapedArray(
                    tuple(alloc.tensor_shape), mybir.dt.np(alloc.dtype)))
        n_params = len(in_names)
        all_names = tuple(in_names) + tuple(out_names)
        if part_name is not None:
            all_names = all_names + (part_name,)
        donate = tuple(range(n_params, n_params + len(out_names)))

        def _body(*args):
            operands = list(args)
            if part_name is not None:
                operands.append(bass2jax.partition_id_tensor())
            outs = bass2jax._bass_exec_p.bind(
                *operands,
                out_avals=tuple(out_avals),
                in_names=all_names,
                out_names=tuple(out_names),
                lowering_input_output_aliases=(),
                sim_require_finite=True,
                sim_require_nnan=True,
                nc=nc,
            )
            return tuple(outs)

        devices = jax.devices()[:N_CORES]
        mesh = Mesh(np.asarray(devices), ("core",))
        spec = PartitionSpec("core")
        fn = jax.jit(
            shard_map(_body, mesh=mesh,
                      in_specs=(spec,) * (n_params + len(out_names)),
                      out_specs=(spec,) * len(out_names), check_rep=False),
            donate_argnums=donate, keep_unused=True)
        zmk = jax.jit(
            lambda: jnp.zeros((N_CORES * BSH, N_CLASSES), ml_dtypes.bfloat16),
            out_shardings=NamedSharding(mesh, spec))
        _RUNNER = (fn, zmk, in_names)
    return _RUNNER


def _patterns():
    """0/±1 combination matrices: C = PC^T M, G = PG^T M (tl = 8t + l)."""
    pcm = np.zeros((512, 64), np.float32)
    pgm = np.zeros((512, 256), np.float32)
    for t in range(N_TREES):
        for l in (0, 2, 4, 6):
            pcm[8 * t + l, t] = 1.0
        for i in range(3):
            pgm[8 * t + 1 + 2 * i, i * 64 + t] = 1.0
            pgm[8 * t + 2 + 2 * i, i * 64 + t] = -1.0
        pgm[8 * t + 7, 3 * 64 + t] = 1.0
    return (pcm.astype(ml_dtypes.bfloat16), pgm.astype(ml_dtypes.bfloat16))


_PC, _PG = _patterns()


def _host_prep(x, tree_params, tree_weights):
    """Slice/layout the replicated params and cast x to bf16 (host-side)."""
    x = np.asarray(x, np.float32).astype(ml_dtypes.bfloat16)
    xt = np.empty((N_CORES * INPUT_DIM, BSH), ml_dtypes.bfloat16)
    for c in range(N_CORES):
        xt[c * INPUT_DIM:(c + 1) * INPUT_DIM] = x[c * BSH:(c + 1) * BSH].T
    p = np.asarray(tree_params, np.float32)[0].reshape(N_TREES, PPT)
    w = p[:, :NW].reshape(N_TREES, N_INTERNAL, INPUT_DIM)[:, :4, :]
    # j = i*64 + t (i-major)
    w_im = np.ascontiguousarray(w.transpose(1, 0, 2).reshape(256, INPUT_DIM))
    wT = np.ascontiguousarray(w_im.T).astype(ml_dtypes.bfloat16)
    bias = np.ascontiguousarray(
        p[:, NW:NW + N_INTERNAL][:, :4].T.reshape(256, 1))
    bias2 = np.ascontiguousarray(bias[192:256] + np.float32(np.log(1.25)))
    ll = np.ascontiguousarray(p[:, NW + N_INTERNAL:].reshape(512, N_CLASSES))
    wr = np.repeat(np.asarray(tree_weights, np.float32)[0], N_LEAVES)
    wr = np.ascontiguousarray(wr.reshape(512, 1))
    return xt, wT, bias, bias2, ll, wr


def kernel(x: np.ndarray, tree_params: np.ndarray,
           tree_weights: np.ndarray) -> np.ndarray:
    fn, zmk, in_names = _get_runner()
    xbf, wT, bias, bias2, ll, wr = _host_prep(x, tree_params, tree_weights)
    reps = {"xt": xbf,
            "wT": np.concatenate([wT] * N_CORES, 0),
            "bb": np.concatenate([bias] * N_CORES, 0),
            "bb2": np.concatenate([bias2] * N_CORES, 0),
            "ll": np.concatenate([ll] * N_CORES, 0),
            "wr": np.concatenate([wr] * N_CORES, 0),
            "pc": np.concatenate([_PC] * N_CORES, 0),
            "pg": np.concatenate([_PG] * N_CORES, 0)}
    args = [reps[n] for n in in_names] + [zmk()]
    outs = fn(*args)
    return np.asarray(outs[0]).astype(np.float32)


# revision 15
# speedup vs baseline: 6044.5359x; 1.1044x over previous
"""Data-parallel GeneratedTreeClassifier forward on 8 NeuronCores (Bass/Tile).

Shards the batch dim of x (16384 -> 8 x 2048) across cores, replicates the
small tree params, runs a hand-written Bass/Tile kernel per core, and
gathers the full [16384, 512] output.

Math restructure (per tree t, decision i = 0..3, r = 1/(4 + d3 + eps)):
  out = leaf_norm @ (softmax(leaf_logits) * w)
      = r @ C + (r*d) @ G
  C_t = M_t0 + M_t2 + M_t4 + M_t6
  G_(t,i) = M_t(1+2i) - M_t(2+2i)  (i<3),   G_(t,3) = M_t7
which kills the per-tile leaf assembly + transpose and shrinks mm2's K
from 512 to 320.

Per-core device graph, processed in groups of 4 batch tiles (512 rows):
  xT   <- PE transpose (identity matmul) of bf16 x tiles  [128d, 4k, 512b]
  zT   = W @ x^T            (PE, j-major: 8 matmuls of N=512)
  d    = sigmoid(zT + bias) (ACT, bias per-partition)     [128j, 2, 512b]
  r    = 1/(4+d3+eps)       (DVE approx, partitions 64:128)
  e    = d * r              (DVE, bf16)   -> mm2 lhsT tiles T0, T1
  out  = [e; r] @ [G; C]    (PE, 3 matmuls of N=512 per batch tile)
  M    = softmax(leaf_logits)*w, C/G via pattern matmuls (one-time).
"""
import numpy as np
import ml_dtypes
from contextlib import ExitStack

import concourse.bass as bass
import concourse.tile as tile
from concourse import bacc, mybir

INPUT_DIM = 512
N_CLASSES = 512
N_TREES = 64
N_LEAVES = 8
N_INTERNAL = 7
PPT = N_INTERNAL * (INPUT_DIM + 1) + N_LEAVES * N_CLASSES
BATCH = 16384
N_CORES = 8
BSH = BATCH // N_CORES          # 2048 rows per core
NB = BSH // 128                 # 16 batch tiles per core
NG = NB // 4                    # 4 groups of 4 tiles
NW = N_INTERNAL * INPUT_DIM
EPS = 1e-8

F32 = mybir.dt.float32
BF16 = mybir.dt.bfloat16


def _emit(ctx: ExitStack, tc, xt, wT, bb, bb2, ll, wr, pc, pg, out):
    nc = tc.nc
    AF = mybir.ActivationFunctionType

    const = ctx.enter_context(tc.tile_pool(name="const", bufs=1))

    # Replicated params, resident in SBUF.
    wt_sb = const.tile([128, 4, 256], BF16)          # [d%128, dk, j]
    nc.sync.dma_start(wt_sb[:], wT.rearrange("(k p) j -> p k j", p=128))
    bias_sb = const.tile([128, 2], F32)              # [j%128, jb]
    nc.sync.dma_start(bias_sb[:], bb.rearrange("(jb p) one -> p (jb one)", p=128))
    bias2_sb = const.tile([128, 1], F32)             # b3 + ln(1.25) at 64:128
    nc.sync.dma_start(bias2_sb[64:128, :], bb2[:])
    pc_sb = const.tile([128, 4, 64], BF16)           # [tl%128, tlk, t]
    nc.sync.dma_start(pc_sb[:], pc.rearrange("(k p) t -> p k t", p=128))
    pg_sb = const.tile([128, 4, 256], BF16)          # [tl%128, tlk, j]
    nc.sync.dma_start(pg_sb[:], pg.rearrange("(k p) j -> p k j", p=128))
    m_sb = const.tile([128, 4, N_CLASSES], BF16)     # [tl%128, tlk, c]
    cg0 = const.tile([128, N_CLASSES], BF16)         # G rows (i0; i1)
    cg1 = const.tile([128, N_CLASSES], BF16)         # G rows (i2; i3)
    cg2 = const.tile([128, N_CLASSES], BF16)         # C rows at 64:128

    # M = softmax(leaf_logits, axis=-1) * w_tree   (rows tl = t*8 + l)
    ppool = ctx.enter_context(tc.tile_pool(name="prm", bufs=2))
    for k in range(4):
        llt = ppool.tile([128, N_CLASSES], F32, tag="llt")
        nc.sync.dma_start(llt[:], ll[k * 128:(k + 1) * 128, :])
        mx = ppool.tile([128, 1], F32, tag="mx")
        nc.vector.reduce_max(mx[:], llt[:], axis=mybir.AxisListType.X)
        nmx = ppool.tile([128, 1], F32, tag="nmx")
        nc.vector.tensor_scalar_mul(nmx[:], mx[:], -1.0)
        e = ppool.tile([128, N_CLASSES], F32, tag="e")
        s = ppool.tile([128, 1], F32, tag="s")
        nc.scalar.activation(e[:], llt[:], AF.Exp,
                             bias=nmx[:], scale=1.0, accum_out=s[:])
        rs = ppool.tile([128, 1], F32, tag="rs")
        nc.vector.reciprocal(rs[:], s[:])
        wrt = ppool.tile([128, 1], F32, tag="wrt")
        nc.sync.dma_start(wrt[:], wr[k * 128:(k + 1) * 128, :])
        sc = ppool.tile([128, 1], F32, tag="sc")
        nc.vector.tensor_tensor(sc[:], rs[:], wrt[:], op=mybir.AluOpType.mult)
        nc.vector.tensor_scalar_mul(m_sb[:, k, :], e[:], sc[:])

    spool = ctx.enter_context(tc.tile_pool(name="xT", bufs=1))
    dpool = ctx.enter_context(tc.tile_pool(name="work", bufs=3))
    epool = ctx.enter_context(tc.tile_pool(name="eT", bufs=3))
    opool = ctx.enter_context(tc.tile_pool(name="osb", bufs=6))
    zpp = ctx.enter_context(tc.tile_pool(name="zps", bufs=4, space="PSUM"))
    opp = ctx.enter_context(tc.tile_pool(name="ops", bufs=4, space="PSUM"))

    # One-time: C/G from M via host-provided 0/±1 pattern matrices.
    cg2ps = opp.tile([128, 512], F32, tag="o")
    for k in range(4):
        nc.tensor.matmul(cg2ps[64:128, :], lhsT=pc_sb[:, k, :],
                         rhs=m_sb[:, k, :], start=(k == 0), stop=(k == 3))
    nc.scalar.copy(cg2[64:128, :], cg2ps[64:128, :])
    cg0ps = opp.tile([128, 512], F32, tag="o")
    for k in range(4):
        nc.tensor.matmul(cg0ps[:], lhsT=pg_sb[:, k, 0:128],
                         rhs=m_sb[:, k, :], start=(k == 0), stop=(k == 3))
    nc.scalar.copy(cg0[:], cg0ps[:])
    cg1ps = opp.tile([128, 512], F32, tag="o")
    for k in range(4):
        nc.tensor.matmul(cg1ps[:], lhsT=pg_sb[:, k, 128:256],
                         rhs=m_sb[:, k, :], start=(k == 0), stop=(k == 3))
    nc.scalar.copy(cg1[:], cg1ps[:])

    # x^T resident in SBUF (pre-transposed on host): xT[p, k, b] = x[b, k*128+p]
    xT = spool.tile([128, 4, BSH], BF16)
    for k, eng in enumerate((nc.sync, nc.scalar, nc.gpsimd, nc.sync)):
        eng.dma_start(xT[:, k, :], xt[k * 128:(k + 1) * 128, :])

    for g in range(NG):
        # zT[j, b] = sum_d W[j, d] x[b, d]    j = i*64 + t, i-major
        gs = slice(g * 512, (g + 1) * 512)
        zt0 = zpp.tile([128, 512], F32, tag="zt")
        zt1 = zpp.tile([128, 512], F32, tag="zt")
        for jb, ztile in enumerate((zt0, zt1)):
            for k in range(4):
                nc.tensor.matmul(ztile[:],
                                 lhsT=wt_sb[:, k, jb * 128:(jb + 1) * 128],
                                 rhs=xT[:, k, gs],
                                 start=(k == 0), stop=(k == 3))
        d4 = dpool.tile([128, 2, 512], BF16, tag="d4")
        for jb, ztile in enumerate((zt0, zt1)):
            nc.scalar.activation(d4[:, jb, :], ztile[:], AF.Sigmoid,
                                 bias=bias_sb[:, jb:jb + 1])

        # r = 1/(4 + d3) = 1/4 - sigmoid(z3 + ln 1.25)/20   (exact identity)
        s3 = dpool.tile([128, 512], F32, tag="s3")
        nc.scalar.activation(s3[64:128, :], zt1[64:128, :], AF.Sigmoid,
                             bias=bias2_sb[64:128, :])
        rb = dpool.tile([128, 512], BF16, tag="rb")
        nc.vector.tensor_scalar(rb[64:128, :], s3[64:128, :], -0.05, 0.25,
                                op0=mybir.AluOpType.mult,
                                op1=mybir.AluOpType.add)
        # replicate r to all (i, jb) lanes:  r4[a*64+t, jb, b] = r[t, b]
        r4 = dpool.tile([128, 2, 512], BF16, tag="r4")
        for jb in range(2):
            for a in range(2):
                nc.gpsimd.dma_start(r4[a * 64:(a + 1) * 64, jb, :],
                                    rb[64:128, :])

        # e = d * r  -> lhsT tiles for mm2 (rows i*64+t match G rows)
        T0 = epool.tile([128, 512], BF16, tag="T0")
        T1 = epool.tile([128, 512], BF16, tag="T1")
        nc.vector.tensor_tensor(T0[:], d4[:, 0, :], r4[:, 0, :],
                                op=mybir.AluOpType.mult)
        nc.vector.tensor_tensor(T1[:], d4[:, 1, :], r4[:, 1, :],
                                op=mybir.AluOpType.mult)

        # out = e @ G + r @ C  per batch tile
        for bt in range(4):
            bs = slice(bt * 128, (bt + 1) * 128)
            ops = opp.tile([128, 512], F32, tag="o")
            nc.tensor.matmul(ops[:], lhsT=T0[:, bs], rhs=cg0[:],
                             start=True, stop=False)
            nc.tensor.matmul(ops[:], lhsT=T1[:, bs], rhs=cg1[:],
                             start=False, stop=False)
            nc.tensor.matmul(ops[:], lhsT=rb[64:128, bs], rhs=cg2[64:128, :],
                             start=False, stop=True)
            osb = opool.tile([128, 512], BF16, tag="osb")
            if bt % 2 == 0:
                nc.scalar.copy(osb[:], ops[:])
            else:
                nc.vector.tensor_copy(osb[:], ops[:])
            deng = nc.sync if bt % 2 == 0 else nc.gpsimd
            deng.dma_start(out[(4 * g + bt) * 128:(4 * g + bt + 1) * 128, :],
                           osb[:])


_NC = None
_RUNNER = None


def _get_nc():
    global _NC
    if _NC is None:
        nc = bacc.Bacc("TRN2", target_bir_lowering=False, debug=False)
        xt = nc.dram_tensor("xt", [INPUT_DIM, BSH], BF16, kind="ExternalInput")
        wT = nc.dram_tensor("wT", [INPUT_DIM, 256], BF16, kind="ExternalInput")
        bb = nc.dram_tensor("bb", [256, 1], F32, kind="ExternalInput")
        bb2 = nc.dram_tensor("bb2", [64, 1], F32, kind="ExternalInput")
        ll = nc.dram_tensor("ll", [512, N_CLASSES], F32, kind="ExternalInput")
        wr = nc.dram_tensor("wr", [512, 1], F32, kind="ExternalInput")
        pc = nc.dram_tensor("pc", [512, 64], BF16, kind="ExternalInput")
        pg = nc.dram_tensor("pg", [512, 256], BF16, kind="ExternalInput")
        out = nc.dram_tensor("out", [BSH, N_CLASSES], BF16, kind="ExternalOutput")
        with tile.TileContext(nc) as tc, ExitStack() as ctx:
            _emit(ctx, tc, xt.ap(), wT.ap(), bb.ap(), bb2.ap(), ll.ap(), wr.ap(),
                  pc.ap(), pg.ap(), out.ap())
        nc.compile()
        _NC = nc
    return _NC


def _get_runner():
    """Build the sharded PJRT executable ONCE (jit + NEFF compile are cached
    across kernel() calls; run_bass_kernel_spmd would re-trace every call)."""
    global _RUNNER
    if _RUNNER is None:
        import jax
        import jax.numpy as jnp
        from jax.sharding import Mesh, PartitionSpec, NamedSharding
        from jax.experimental.shard_map import shard_map
        from concourse import bass2jax

        nc = _get_nc()
        bass2jax.install_neuronx_cc_hook()

        part_name = (nc.partition_id_tensor.name
                     if nc.partition_id_tensor else None)
        in_names, out_names, out_avals = [], [], []
        for alloc in nc.m.functions[0].allocations:
            if not isinstance(alloc, mybir.MemoryLocationSet):
                continue
            name = alloc.memorylocations[0].name
            if alloc.kind == "ExternalInput":
                if name != part_name:
                    in_names.append(name)
            elif alloc.kind == "ExternalOutput":
                out_names.append(name)
                out_avals.append(jax.core.ShapedArray(
                    tuple(alloc.tensor_shape), mybir.dt.np(alloc.dtype)))
        n_params = len(in_names)
        all_names = tuple(in_names) + tuple(out_names)
        if part_name is not None:
            all_names = all_names + (part_name,)
        donate = tuple(range(n_params, n_params + len(out_names)))

        def _body(*args):
            operands = list(args)
            if part_name is not None:
                operands.append(bass2jax.partition_id_tensor())
            outs = bass2jax._bass_exec_p.bind(
                *operands,
                out_avals=tuple(out_avals),
                in_names=all_names,
                out_names=tuple(out_names),
                lowering_input_output_aliases=(),
                sim_require_finite=True,
                sim_require_nnan=True,
                nc=nc,
            )
            return tuple(outs)

        devices = jax.devices()[:N_CORES]
        mesh = Mesh(np.asarray(devices), ("core",))
        spec = PartitionSpec("core")
        fn = jax.jit(
            shard_map(_body, mesh=mesh,
                      in_specs=(spec,) * (n_params + len(out_names)),
                      out_specs=(spec,) * len(out_names), check_rep=False),
            donate_argnums=donate, keep_unused=True)
        zmk = jax.jit(
            lambda: jnp.zeros((N_CORES * BSH, N_CLASSES), ml_dtypes.bfloat16),
            out_shardings=NamedSharding(mesh, spec))
        _RUNNER = (fn, zmk, in_names)
    return _RUNNER


def _patterns():
    """0/±1 combination matrices: C = PC^T M, G = PG^T M (tl = 8t + l)."""
    pcm = np.zeros((512, 64), np.float32)
    pgm = np.zeros((512, 256), np.float32)
    for t in range(N_TREES):
        for l in (0, 2, 4, 6):
            pcm[8 * t + l, t] = 1.0
        for i in range(3):
            pgm[8 * t + 1 + 2 * i, i * 64 + t] = 1.0
            pgm[8 * t + 2 + 2 * i, i * 64 + t] = -1.0
        pgm[8 * t + 7, 3 * 64 + t] = 1.0
    return (pcm.astype(ml_dtypes.bfloat16), pgm.astype(ml_dtypes.bfloat16))


_PC, _PG = _patterns()


def _host_prep(x, tree_params, tree_weights):
    """Slice/layout the replicated params and cast x to bf16 (host-side)."""
    x = np.asarray(x, np.float32).astype(ml_dtypes.bfloat16)
    xt = np.empty((N_CORES * INPUT_DIM, BSH), ml_dtypes.bfloat16)
    for c in range(N_CORES):
        xt[c * INPUT_DIM:(c + 1) * INPUT_DIM] = x[c * BSH:(c + 1) * BSH].T
    p = np.asarray(tree_params, np.float32)[0].reshape(N_TREES, PPT)
    w = p[:, :NW].reshape(N_TREES, N_INTERNAL, INPUT_DIM)[:, :4, :]
    # j = i*64 + t (i-major)
    w_im = np.ascontiguousarray(w.transpose(1, 0, 2).reshape(256, INPUT_DIM))
    wT = np.ascontiguousarray(w_im.T).astype(ml_dtypes.bfloat16)
    bias = np.ascontiguousarray(
        p[:, NW:NW + N_INTERNAL][:, :4].T.reshape(256, 1))
    bias2 = np.ascontiguousarray(bias[192:256] + np.float32(np.log(1.25)))
    ll = np.ascontiguousarray(p[:, NW + N_INTERNAL:].reshape(512, N_CLASSES))
    wr = np.repeat(np.asarray(tree_weights, np.float32)[0], N_LEAVES)
    wr = np.ascontiguousarray(wr.reshape(512, 1))
    return xt, wT, bias, bias2, ll, wr


def kernel(x: np.ndarray, tree_params: np.ndarray,
           tree_weights: np.ndarray) -> np.ndarray:
    fn, zmk, in_names = _get_runner()
    xbf, wT, bias, bias2, ll, wr = _host_prep(x, tree_params, tree_weights)
    reps = {"xt": xbf,
            "wT": np.concatenate([wT] * N_CORES, 0),
            "bb": np.concatenate([bias] * N_CORES, 0),
            "bb2": np.concatenate([bias2] * N_CORES, 0),
            "ll": np.concatenate([ll] * N_CORES, 0),
            "wr": np.concatenate([wr] * N_CORES, 0),
            "pc": np.concatenate([_PC] * N_CORES, 0),
            "pg": np.concatenate([_PG] * N_CORES, 0)}
    args = [reps[n] for n in in_names] + [zmk()]
    outs = fn(*args)
    return np.asarray(outs[0]).astype(np.float32)
